# revision 45
# baseline (speedup 1.0000x reference)
"""Trainium2 Bass kernel for nn_ClassificationLoss (NMS-detection CE loss).

Data-parallel across 8 NeuronCores: each core handles 2 of the 16 images.

Fully chunked pipeline (KB preds per chunk, double-buffered scratch):
  - Pairwise IoU selector in fp16; GT fields replicated along the innermost
    pred dim so tensor_tensor ops hit the DVE 2x perf mode. Validity uses the
    division-free form  valid <=> max_m (inter - th*(pa+ga)) >= 0, with the
    y-axis pre-scaled by 1/8 to keep fp16 in range.
  - Label = gcls[argmax_m u] via eq-against-min + single-nonzero add-tree.
  - CE: exp on the Act engine written transposed ([P, c, k] chunk), fp16
    halving add-tree for sum(exp), radix-(16x5) masked add-tree gather of
    exp(s)[label]; ce = ln(se) - ln(exp(s_label)).
  - Per-partition masked sums -> [126, 4] partials; host does the tiny scalar
    epilogue.

Engine placement (DVE "v" / Pool "p") is in ENG; Pool only supports
add/subtract/mult through neuronxcc.
"""

import numpy as np

import concourse.bass as bass
import concourse.bacc as bacc
import concourse.tile as tile
import concourse.mybir as mybir
from concourse.bass_utils import run_bass_kernel_spmd

B, N, C, M = 16, 25200, 80, 64
NCORES = 8
IMGS_PER_CORE = B // NCORES          # 2
P = 126                              # partitions; 126 * 200 = 25200
R = N // P                           # 200 preds per partition per image
KB = 25                              # preds per chunk
NCH = R // KB                        # 8 chunks per image
TH = float(np.float32(2.0) / np.float32(7.0))
YS = 0.125                           # y-axis scale to keep fp16 in range

F32 = mybir.dt.float32
F16 = mybir.dt.float16
Alu = mybir.AluOpType
Act = mybir.ActivationFunctionType
AX = mybir.AxisListType

_CACHE = {}

# engine per op: "v" = DVE vector, "p" = Pool/gpsimd (add/sub/mult only!)
ENG = {
    "wn": "v", "hn": "v", "i0": "v", "un": "v", "lw": "p",
    "lt": "p",                        # label add-tree
    "se": "p",                        # se add-tree
    "s1": "p", "lotree": "p",         # radix stage 1
    "m2": "p", "hitree": "p",
}


def _bc(ap_like, extra_offset, dims):
    """Raw AP with explicit [step, count] dims (0-step = broadcast)."""
    return bass.AP(tensor=ap_like.tensor, offset=ap_like.offset + extra_offset, ap=dims)


def _build():
    nc = bacc.Bacc("TRN2")
    pb_in = nc.dram_tensor("pb", [IMGS_PER_CORE, N, 4], F32, kind="ExternalInput")
    ps_in = nc.dram_tensor("ps", [IMGS_PER_CORE, N, C], F32, kind="ExternalInput")
    g_in = nc.dram_tensor("g", [IMGS_PER_CORE, M, 5], F32, kind="ExternalInput")
    o_out = nc.dram_tensor("o", [P, 2 * IMGS_PER_CORE], F32, kind="ExternalOutput")

    def eng(key):
        return nc.gpsimd if ENG[key] == "p" else nc.vector

    with tile.TileContext(nc) as tc:
        with (
            tc.tile_pool(name="singles", bufs=1) as singles,
            tc.tile_pool(name="imgp", bufs=1) as imgp,
            tc.tile_pool(name="accp", bufs=2) as accp,
            tc.tile_pool(name="scp", bufs=3) as scp,
            tc.tile_pool(name="scr", bufs=2) as scr,
        ):
            # ---------- one-time constants: iotalo [P,16,KB], iotahi [P,5,KB]
            def const_idx_tile(n, name):
                it = singles.tile([P, n], mybir.dt.int32, tag=f"{name}_i")
                nc.gpsimd.iota(it, pattern=[[1, n]], base=0, channel_multiplier=0)
                fh = singles.tile([P, n], F16, tag=f"{name}_h")
                nc.vector.tensor_copy(fh, it)
                t = singles.tile([P, n, R], F16, tag=name)
                ta = t[:, :, :]
                fa = fh[:, :]
                nc.vector.tensor_copy(
                    _bc(ta, 0, [ta.ap[0], [R, n], [1, 1]]),
                    _bc(fa, 0, [fa.ap[0], [1, n], [1, 1]]),
                )
                w = 1
                while w < R:
                    cw = min(w, R - w)
                    nc.vector.tensor_copy(
                        _bc(ta, w, [ta.ap[0], [R, n], [1, cw]]),
                        _bc(ta, 0, [ta.ap[0], [R, n], [1, cw]]),
                    )
                    w += cw
                return t

            iotalo = const_idx_tile(16, "iotalo")
            iotahi = const_idx_tile(5, "iotahi")

            out_t = singles.tile([P, 2 * IMGS_PER_CORE], F32)

            for b in range(IMGS_PER_CORE):
                # ---------- GT prep (DRAM broadcast to all partitions) ----
                graw = imgp.tile([P, M, 5], F32, tag="graw")
                nc.sync.dma_start(
                    out=graw,
                    in_=_bc(g_in[:], b * M * 5, [[0, P], [5, M], [1, 5]]))

                def gcol(col):
                    a = graw[:, :, :]
                    return _bc(a, col, [a.ap[0], [5, M], [1, 1]])

                grows = {}
                for name, col, scale in (
                    ("gx1", 0, 1.0), ("ngx2", 2, -1.0),
                    ("gy1", 1, YS), ("ngy2", 3, -YS),
                ):
                    t = imgp.tile([P, M], F16, tag=f"gr_{name}")
                    ta = t[:, :]
                    nc.vector.tensor_scalar(
                        _bc(ta, 0, [ta.ap[0], [1, M], [1, 1]]),
                        gcol(col), scale, None, op0=Alu.mult)
                    grows[name] = t
                gcl16 = imgp.tile([P, M], F16, tag="gr_gcl")
                ta = gcl16[:, :]
                nc.vector.tensor_copy(_bc(ta, 0, [ta.ap[0], [1, M], [1, 1]]), gcol(4))
                d1 = imgp.tile([P, M], F32, tag="gr_d1")
                d2 = imgp.tile([P, M], F32, tag="gr_d2")
                ga = imgp.tile([P, M], F32, tag="gr_ga")
                nc.vector.tensor_tensor(
                    _bc(d1[:, :], 0, [d1[:, :].ap[0], [1, M], [1, 1]]),
                    gcol(2), gcol(0), op=Alu.subtract)
                nc.vector.tensor_tensor(
                    _bc(d2[:, :], 0, [d2[:, :].ap[0], [1, M], [1, 1]]),
                    gcol(3), gcol(1), op=Alu.subtract)
                nc.vector.tensor_tensor(ga, d1, d2, op=Alu.mult)
                tga8 = imgp.tile([P, M], F16, tag="gr_tga8")
                nc.vector.tensor_scalar(tga8, ga[:, :], TH * YS, None, op0=Alu.mult)
                grows["tga8"] = tga8
                grows["gcl"] = gcl16

                # replicated GT tiles [P, M, KB] via doubling copies
                grep = {}
                for name in ("gx1", "ngx2", "gy1", "ngy2", "tga8", "gcl"):
                    t = imgp.tile([P, M, KB], F16, tag=f"rep_{name}")
                    ta = t[:, :, :]
                    ra = grows[name][:, :]
                    nc.vector.tensor_copy(
                        _bc(ta, 0, [ta.ap[0], [KB, M], [1, 1]]),
                        _bc(ra, 0, [ra.ap[0], [1, M], [1, 1]]),
                    )
                    w = 1
                    while w < KB:
                        cw = min(w, KB - w)
                        nc.vector.tensor_copy(
                            _bc(ta, w, [ta.ap[0], [KB, M], [1, cw]]),
                            _bc(ta, 0, [ta.ap[0], [KB, M], [1, cw]]),
                        )
                        w += cw
                    grep[name] = t

                def grb(name):
                    return grep[name][:, :, :]

                # ---------- pred field prep ----------
                boxes = imgp.tile([P, R, 4], F32, tag="boxes")
                nc.sync.dma_start(
                    out=boxes, in_=pb_in[b].rearrange("(p r) c -> p r c", p=P))

                def bxc(col):
                    a = boxes[:, :, :]
                    return _bc(a, col, [a.ap[0], [4, R], [1, 1]])

                pf = {}
                for name, col, scale in (
                    ("px1", 0, 1.0), ("npx2", 2, -1.0),
                    ("py1", 1, YS), ("npy2", 3, -YS),
                ):
                    t = imgp.tile([P, R], F16, tag=f"pf_{name}")
                    ta = t[:, :]
                    nc.scalar.activation(
                        _bc(ta, 0, [ta.ap[0], [1, R], [1, 1]]),
                        bxc(col), Act.Copy, bias=0.0, scale=scale)
                    pf[name] = t
                wx = imgp.tile([P, R], F32, tag="pf_wx")
                wy = imgp.tile([P, R], F32, tag="pf_wy")
                pa = imgp.tile([P, R], F32, tag="pf_pa")
                nc.vector.tensor_tensor(
                    _bc(wx[:, :], 0, [wx[:, :].ap[0], [1, R], [1, 1]]),
                    bxc(2), bxc(0), op=Alu.subtract)
                nc.vector.tensor_tensor(
                    _bc(wy[:, :], 0, [wy[:, :].ap[0], [1, R], [1, 1]]),
                    bxc(3), bxc(1), op=Alu.subtract)
                nc.vector.tensor_tensor(pa, wx, wy, op=Alu.mult)
                tpa8 = imgp.tile([P, R], F16, tag="pf_tpa8")
                nc.vector.tensor_scalar(tpa8, pa[:, :], TH * YS, None, op0=Alu.mult)

                def pfb(name, c0):  # pred field chunk bcast over M
                    a = pf[name][:, :]
                    return _bc(a, c0, [a.ap[0], [0, M], [1, KB]])

                cev = accp.tile([P, R], F32, tag="cev")
                vacc = accp.tile([P, R], F32, tag="vacc")
                unminb = accp.tile([P, R], F16, tag="unminb")
                labb = accp.tile([P, R], F16, tag="labb")
                escT = imgp.tile([P, C, R], F16, tag="escT")

                # ---------- unified chunk loop ----------
                for ci in range(NCH):
                    c0 = ci * KB
                    # --- class input: scores chunk -> exp (transposed out)
                    sch = scp.tile([P, KB, C], F32, tag="sch")
                    nc.sync.dma_start(
                        out=sch,
                        in_=_bc(ps_in[:], (b * N + c0) * C,
                                [[R * C, P], [C, KB], [1, C]]))
                    ea = escT[:, :, :]
                    nc.scalar.activation(
                        _bc(ea, c0, [ea.ap[0], [1, KB], [R, C]]),
                        sch[:, :, :], Act.Exp)

                    # --- pairwise chain
                    t1 = scr.tile([P, M, KB], F16, tag="A")
                    t2n = scr.tile([P, M, KB], F16, tag="B")
                    wn = scr.tile([P, M, KB], F16, tag="wn")
                    t3 = scr.tile([P, M, KB], F16, tag="C")
                    t4n = scr.tile([P, M, KB], F16, tag="D")
                    hn = scr.tile([P, M, KB], F16, tag="hn")
                    nc.vector.tensor_tensor(t1, grb("gx1"), pfb("px1", c0), op=Alu.max)
                    nc.vector.tensor_tensor(t2n, grb("ngx2"), pfb("npx2", c0), op=Alu.max)
                    nc.vector.tensor_tensor(t3, grb("gy1"), pfb("py1", c0), op=Alu.max)
                    nc.vector.tensor_tensor(t4n, grb("ngy2"), pfb("npy2", c0), op=Alu.max)
                    eng("wn").tensor_tensor(wn, t1, t2n, op=Alu.add)
                    eng("hn").tensor_tensor(hn, t3, t4n, op=Alu.add)
                    r_ = scr.tile([P, M, KB], F16, tag="A")
                    nc.scalar.activation(r_, wn, Act.Relu, bias=0.0, scale=-1.0)
                    i0 = scr.tile([P, M, KB], F16, tag="C")
                    eng("i0").tensor_tensor(i0, r_, hn, op=Alu.mult)  # -relu(w)h/8
                    un = scr.tile([P, M, KB], F16, tag="un")
                    eng("un").tensor_tensor(un, grb("tga8"), i0, op=Alu.add)

                    # min-tree over M
                    unm = unminb[:, c0:c0 + KB]
                    cur = un
                    m = M
                    while m > 1:
                        h = m // 2
                        if h > 1:
                            nxt = scr.tile([P, h, KB], F16, tag=f"mt{h}")
                            nc.vector.tensor_tensor(
                                nxt, cur[:, 0:h, :], cur[:, h:m, :], op=Alu.min)
                            cur = nxt
                        else:
                            nc.vector.tensor_tensor(
                                unm, cur[:, 0:1, :].rearrange("p a k -> p (a k)"),
                                cur[:, 1:2, :].rearrange("p a k -> p (a k)"), op=Alu.min)
                        m = h

                    # label: eq + gcls mult + add-tree (single nonzero)
                    equn = scr.tile([P, M, KB], F16, tag="B")
                    ua = unm
                    nc.vector.tensor_tensor(
                        equn, un, _bc(ua, 0, [ua.ap[0], [0, M], [1, KB]]),
                        op=Alu.is_equal)
                    lw = scr.tile([P, M, KB], F16, tag="D")
                    eng("lw").tensor_tensor(lw, equn, grb("gcl"), op=Alu.mult)
                    lab = labb[:, c0:c0 + KB]
                    cur = lw
                    m = M
                    while m > 1:
                        h = m // 2
                        if h > 1:
                            nxt = scr.tile([P, h, KB], F16, tag=f"lt{h}")
                            eng("lt").tensor_tensor(
                                nxt, cur[:, 0:h, :], cur[:, h:m, :], op=Alu.add)
                            cur = nxt
                        else:
                            eng("lt").tensor_tensor(
                                lab, cur[:, 0:1, :].rearrange("p a k -> p (a k)"),
                                cur[:, 1:2, :].rearrange("p a k -> p (a k)"), op=Alu.add)
                        m = h

                # ---------- image-level class part ----------
                nc.vector.tensor_scalar(labb, labb[:, :], 79.0, None, op0=Alu.min)

                # se = sum_c escT (fp16 add-tree, chunked over k)
                seb = accp.tile([P, R], F16, tag="seb")
                for ci in range(NCH):
                    c0 = ci * KB
                    cur = None
                    m = C
                    while m > 1:
                        h = m // 2
                        if cur is None:
                            nxt = scr.tile([P, h, KB], F16, tag=f"se{h}")
                            ea_ = escT[:, :, :]
                            nc.vector.tensor_tensor(
                                nxt,
                                _bc(ea_, c0, [ea_.ap[0], [R, h], [1, KB]]),
                                _bc(ea_, c0 + h * R, [ea_.ap[0], [R, h], [1, KB]]),
                                op=Alu.add)
                            cur = nxt
                        elif h > 1:
                            nxt = scr.tile([P, h, KB], F16, tag=f"se{h}")
                            nc.vector.tensor_tensor(
                                nxt, cur[:, 0:h, :], cur[:, h:2 * h, :], op=Alu.add)
                            if m % 2 == 1:
                                nc.vector.tensor_tensor(
                                    nxt[:, 0:1, :], nxt[:, 0:1, :], cur[:, 2 * h:m, :],
                                    op=Alu.add)
                            cur = nxt
                        else:
                            nc.vector.tensor_tensor(
                                seb[:, c0:c0 + KB],
                                cur[:, 0:1, :].rearrange("p a k -> p (a k)"),
                                cur[:, 1:2, :].rearrange("p a k -> p (a k)"), op=Alu.add)
                        m = h

                # radix split of label (image level)
                g1 = imgp.tile([P, R], F16, tag="g1")
                g2 = imgp.tile([P, R], F16, tag="g2")
                g3 = imgp.tile([P, R], F16, tag="g3")
                g4 = imgp.tile([P, R], F16, tag="g4")
                nc.vector.tensor_scalar(g1, labb[:, :], 16.0, None, op0=Alu.is_ge)
                nc.vector.tensor_scalar(g2, labb[:, :], 32.0, None, op0=Alu.is_ge)
                nc.vector.tensor_scalar(g3, labb[:, :], 48.0, None, op0=Alu.is_ge)
                nc.vector.tensor_scalar(g4, labb[:, :], 64.0, None, op0=Alu.is_ge)
                ga_ = imgp.tile([P, R], F16, tag="ga_")
                gb_ = imgp.tile([P, R], F16, tag="gb_")
                nc.vector.tensor_tensor(ga_, g1, g2, op=Alu.add)
                nc.vector.tensor_tensor(gb_, g3, g4, op=Alu.add)
                hi = imgp.tile([P, R], F16, tag="hi")
                nc.vector.tensor_tensor(hi, ga_, gb_, op=Alu.add)
                n16 = imgp.tile([P, R], F16, tag="n16")
                nc.vector.tensor_scalar(n16, hi[:, :], -16.0, None, op0=Alu.mult)
                lo = imgp.tile([P, R], F16, tag="lo")
                nc.vector.tensor_tensor(lo, labb, n16, op=Alu.add)

                # masked radix gather of exp(s)[label]
                masklo = imgp.tile([P, 16, R], F16, tag="masklo")
                loa = lo[:, :]
                nc.vector.tensor_tensor(
                    masklo,
                    _bc(loa, 0, [loa.ap[0], [0, 16], [1, R]]),
                    iotalo[:, :, :], op=Alu.is_equal)
                m1 = imgp.tile([P, 5, R], F16, tag="m1")
                for hi_i in range(5):
                    s1 = scr.tile([P, 16, R], F16, tag="s1")
                    nc.vector.tensor_tensor(
                        s1, escT[:, hi_i * 16:(hi_i + 1) * 16, :], masklo, op=Alu.mult)
                    cur = s1
                    m = 16
                    while m > 1:
                        h = m // 2
                        if h > 1:
                            nxt = scr.tile([P, h, R], F16, tag=f"rlo{h}")
                            nc.vector.tensor_tensor(
                                nxt, cur[:, 0:h, :], cur[:, h:m, :], op=Alu.add)
                            cur = nxt
                        else:
                            nc.vector.tensor_tensor(
                                m1[:, hi_i:hi_i + 1, :],
                                cur[:, 0:1, :], cur[:, 1:2, :], op=Alu.add)
                        m = h
                eqhi = imgp.tile([P, 5, R], F16, tag="eqhi")
                hia = hi[:, :]
                nc.vector.tensor_tensor(
                    eqhi,
                    _bc(hia, 0, [hia.ap[0], [0, 5], [1, R]]),
                    iotahi[:, :, :], op=Alu.is_equal)
                m2 = imgp.tile([P, 5, R], F16, tag="m2")
                nc.vector.tensor_tensor(m2, m1, eqhi, op=Alu.mult)
                m2a = scr.tile([P, 2, R], F16, tag="m2a")
                nc.vector.tensor_tensor(m2a, m2[:, 0:2, :], m2[:, 2:4, :], op=Alu.add)
                m2b = scr.tile([P, 1, R], F16, tag="m2b")
                nc.vector.tensor_tensor(m2b, m2a[:, 0:1, :], m2a[:, 1:2, :], op=Alu.add)
                esclab = imgp.tile([P, R], F16, tag="esclab")
                nc.vector.tensor_tensor(
                    esclab, m2b[:, 0:1, :].rearrange("p a k -> p (a k)"),
                    m2[:, 4:5, :].rearrange("p a k -> p (a k)"), op=Alu.add)
                nc.vector.tensor_scalar(esclab, esclab[:, :], 1e-4, None, op0=Alu.max)

                # lse, slab, validity, ce
                lse32 = imgp.tile([P, R], F32, tag="lse32")
                nc.scalar.activation(lse32, seb, Act.Ln)
                slab32 = imgp.tile([P, R], F32, tag="slab32")
                nc.scalar.activation(slab32, esclab, Act.Ln)
                nunm = imgp.tile([P, R], F16, tag="nunm")
                nc.vector.tensor_scalar(nunm, unminb[:, :], -1.0, None, op0=Alu.mult)
                nc.vector.tensor_tensor(vacc, nunm, tpa8, op=Alu.is_ge)
                cech = imgp.tile([P, R], F32, tag="cech")
                nc.vector.tensor_tensor(cech, lse32, slab32, op=Alu.subtract)
                nc.vector.tensor_tensor(cev, cech, vacc, op=Alu.mult)

                # ---------- per-image reduction ----------
                nc.vector.tensor_reduce(
                    out_t[:, 2 * b:2 * b + 1], cev[:, :], axis=AX.X, op=Alu.add)
                nc.vector.tensor_reduce(
                    out_t[:, 2 * b + 1:2 * b + 2], vacc[:, :], axis=AX.X, op=Alu.add)

            nc.sync.dma_start(out=o_out[:], in_=out_t)

    nc.compile()
    return nc


def kernel(preds: np.ndarray, gtruths: np.ndarray) -> np.ndarray:
    if "nc" not in _CACHE:
        _CACHE["nc"] = _build()
    nc = _CACHE["nc"]

    preds = np.ascontiguousarray(preds, dtype=np.float32)
    gtruths = np.ascontiguousarray(gtruths, dtype=np.float32)
    in_maps = []
    for c in range(NCORES):
        sl = slice(c * IMGS_PER_CORE, (c + 1) * IMGS_PER_CORE)
        in_maps.append({
            "pb": np.ascontiguousarray(preds[sl, :, 0:4]),
            "ps": np.ascontiguousarray(preds[sl, :, 5:85]),
            "g": np.ascontiguousarray(gtruths[sl]),
        })
    res = run_bass_kernel_spmd(nc, in_maps, core_ids=list(range(NCORES)))
    _CACHE["last_result"] = res

    per_img = []
    for c in range(NCORES):
        o = res.results[c]["o"]  # [P, 4]
        for b in range(IMGS_PER_CORE):
            ce_sum = float(o[:, 2 * b].sum(dtype=np.float64))
            cnt = float(o[:, 2 * b + 1].sum(dtype=np.float64))
            per_img.append(ce_sum / max(cnt, 1.0))
    return np.asarray(np.mean(per_img), dtype=np.float32)
